# revision 12
# baseline (speedup 1.0000x reference)
"""CapsuleLayer routing kernel for 8 Trainium2 NeuronCores.

Problem (full shapes): x [B=32, N=2048, IC=16] fp32,
route_weights [N=2048, K=32, IC=16, OC=32] fp32.
  priors = einsum('bni,nkio->bnko', x, W)
  3 routing iterations (softmax over K, weighted sum over N, squash)
  output = squash(s2) shaped [B, 1, K, 1, OC].

Sharding: N (nodes) sharded 8 ways (256 nodes/core).  Cross-core
traffic: one bf16 AllReduce of s [B, K*OC] (64KB) per non-final
iteration; the final iteration's local fp32 s2 partial is DMAed out and
the host sums the 8 partials + applies squash (gather/unshard of the
sum-sharded result).

v5 structure (measured-informed):
  - W is STREAMED in 8 x 1MB bf16 chunks alternating between the SP
    and ACT DMA queues (two queues overlap transfers; ~300GB/s
    aggregate measured vs ~184 single-queue).  The chunk sweep feeds
    the s0 matmuls; the s0 AllReduce fires right after the last chunk.
  - priors are materialized ONCE into 16 persistent SBUF tiles
    (128KB/partition, bf16): supertiles 14,15 during the s0-AllReduce
    window from the still-resident last chunk, 0,1 from a reload, the
    rest interleaved into pass B with 2-slot lookahead.  Pass C reads
    priors straight from SBUF (DVE-bound, no PE cost).
  - DVE per-supertile slot (the hard wall, ~0.555ns/elem tensor_tensor
    in 2x mode; TENSOR_REDUCE measured SLOWER at 1.1-1.8ns/elem, so
    the 4-level bf16 add-tree stays): tt (P*vrep) + tree + 1/Z
    reciprocal + wp (exp*P).  Z comes free from the exp ACTIVATE's
    accum_out (per group).
"""

import numpy as np
import ml_dtypes

B, NLOC, K, IC, OC = 32, 256, 32, 16, 32
NCORES = 8
N = NLOC * NCORES
KO = K * OC            # 1024
NT = NLOC // 8         # 32 sub-tiles of 8 nodes
NST = NLOC // 16       # 16 supertiles of 16 nodes (4 groups of 4)
NGRP = NLOC // 4       # 64 groups of 4 nodes
NCH = 8                # W chunks (2 supertiles per chunk)

_CACHE = {}


def _build_bass():
    import concourse.bass as bass
    import concourse.mybir as mybir
    from concourse import bacc, tile

    dt = mybir.dt
    AF = mybir.ActivationFunctionType
    ALU = mybir.AluOpType

    nc = bacc.Bacc("TRN2", target_bir_lowering=False)

    wmov_d = nc.declare_dram_parameter("wmov", [128, NT * KO], dt.bfloat16, isOutput=False)
    xblk_d = nc.declare_dram_parameter("xblk", [128, NT * 128], dt.bfloat16, isOutput=False)
    xall_d = nc.declare_dram_parameter("xall", [128, NT * B], dt.bfloat16, isOutput=False)
    ones_d = nc.declare_dram_parameter("onesblk", [128, B], dt.bfloat16, isOutput=False)
    vout_d = nc.declare_dram_parameter("vout", [B, KO], dt.float32, isOutput=True)

    groups = [list(range(NCORES))]
    CHW = 4 * KO  # W chunk: 2 supertiles (4 subtiles)

    with tile.TileContext(nc) as tc:
        with (
            tc.tile_pool(name="wch", bufs=2) as wpool,
            tc.tile_pool(name="persist", bufs=1) as ppool,
            tc.tile_pool(name="ptile", bufs=1) as p_pool,
            tc.tile_pool(name="ltiles", bufs=1) as lpool,
            tc.tile_pool(name="tsb", bufs=1) as t_pool,
            tc.tile_pool(name="tree", bufs=1) as u_pool,
            tc.tile_pool(name="wp", bufs=2) as wp_pool,
            tc.tile_pool(name="eexp", bufs=1) as e_pool,
            tc.tile_pool(name="sm", bufs=2) as sm_pool,
            tc.tile_pool(name="vv", bufs=1) as v_pool,
            tc.tile_pool(name="ppsum", bufs=3, space="PSUM") as ppsum_pool,
            tc.tile_pool(name="spsum", bufs=1, space="PSUM") as spsum_pool,
            tc.tile_pool(name="dram", bufs=4, space="DRAM") as dram_pool,
        ):
            xblk = ppool.tile([128, NT * 128], dt.bfloat16, tag="xblk")
            xall = ppool.tile([128, NT * B], dt.bfloat16, tag="xall")
            onesblk = ppool.tile([128, B], dt.bfloat16, tag="ones")

            # CC warmup: tiny AllReduce issued first (content irrelevant),
            # absorbs the ~48us collective-stack cold start under the loads.
            warm_in = dram_pool.tile([1, 4], dt.float32, tag="warmin")
            warm_out = dram_pool.tile([1, 4], dt.float32, tag="warmout")
            nc.gpsimd.collective_compute(
                "AllReduce", ALU.add, replica_groups=groups,
                ins=[warm_in.opt()], outs=[warm_out.opt()],
            )
            warm_in2 = dram_pool.tile([1, 4], dt.float32, tag="warmin2")
            warm_out2 = dram_pool.tile([1, 4], dt.float32, tag="warmout2")
            nc.gpsimd.collective_compute(
                "AllReduce", ALU.add, replica_groups=groups,
                ins=[warm_in2.opt()], outs=[warm_out2.opt()],
            )

            nc.sync.dma_start(out=xall[:], in_=xall_d[:])
            nc.scalar.dma_start(out=onesblk[:], in_=ones_d[:])

            # persistent priors tiles, one [128(4n,32b), 4*KO] per supertile
            ptiles = [p_pool.tile([128, 4 * KO], dt.bfloat16, tag=f"P{t}",
                                  name=f"P{t}") for t in range(NST)]
            # persistent logits tiles, one [128(4n,32b), 4*K] per supertile
            ltiles = [lpool.tile([128, 4 * K], dt.float32, tag=f"L{t}",
                                 name=f"L{t}") for t in range(NST)]

            def load_wchunk(c, eng=None):
                wc = wpool.tile([128, CHW], dt.bfloat16, tag="wc",
                                name=f"wc{c}")
                (eng or nc.sync).dma_start(
                    out=wc[:], in_=wmov_d[:, c * CHW:(c + 1) * CHW])
                return wc

            def materialize(st, wc, half):
                """pp matmuls + psb copy for supertile st.

                wc holds chunk st//2; half = st % 2 selects its supertile.
                """
                for tsub in (0, 1):
                    t = 2 * st + tsub
                    for s in (0, 1):
                        g = 2 * tsub + s
                        pp = ppsum_pool.tile(
                            [128, KO], dt.float32, tag="pp",
                            name=f"pp{st}_{g}")
                        lhs = xblk[s * 64:(s + 1) * 64,
                                   t * 128:(t + 1) * 128]
                        co = (2 * half + tsub) * KO
                        for h in (0, 1):
                            nc.tensor.matmul(
                                out=pp[:, h * 512:(h + 1) * 512], lhsT=lhs,
                                rhs=wc[s * 64:(s + 1) * 64,
                                       co + h * 512:co + (h + 1) * 512],
                                start=True, stop=True,
                                skip_group_check=True,
                            )
                        nc.scalar.copy(
                            out=ptiles[st][:, g * KO:(g + 1) * KO], in_=pp[:])

            def allreduce_squash(s_ps0, s_ps1, last, it):
                """PSUM s halves -> AllReduce(bf16) -> squash -> vrep tile.

                Free layout everywhere is (o,k): s[b, o*K + k].
                On the last iteration the local fp32 partial is DMAed out
                instead: the host sums the 8 partials and applies squash
                (gather/unshard of the sum-sharded result).
                """
                if last:
                    sh0 = v_pool.tile([B, 512], dt.float32, tag="sfh",
                                      name=f"sfh{it}_0", bufs=1)
                    nc.scalar.copy(out=sh0[:], in_=s_ps0[:])
                    nc.gpsimd.dma_start(out=vout_d[:, 0:512], in_=sh0[:])
                    sh1 = v_pool.tile([B, 512], dt.float32, tag="sfh",
                                      name=f"sfh{it}_1", bufs=1)
                    nc.scalar.copy(out=sh1[:], in_=s_ps1[:])
                    nc.gpsimd.dma_start(out=vout_d[:, 512:1024], in_=sh1[:])
                    return None
                sfull = v_pool.tile([B, KO], dt.bfloat16, tag="sfull",
                                    name=f"sfull{it}", bufs=1)
                nrm = v_pool.tile([B, K], dt.float32, tag="nrm",
                                  name=f"nrm{it}", bufs=1)
                nrm1 = v_pool.tile([B, K], dt.float32, tag="nrm1",
                                   name=f"nrm1{it}", bufs=1)
                den = v_pool.tile([B, K], dt.float32, tag="den",
                                  name=f"den{it}", bufs=1)
                rden = v_pool.tile([B, K], dt.float32, tag="rden",
                                   name=f"rden{it}", bufs=1)
                nc.scalar.copy(out=sfull[:, 0:512], in_=s_ps0[:])
                nc.scalar.copy(out=sfull[:, 512:1024], in_=s_ps1[:])
                cc_in = dram_pool.tile([B, KO], dt.bfloat16, tag="ccin")
                cc_out = dram_pool.tile([B, KO], dt.bfloat16, tag="ccout")
                nc.gpsimd.dma_start(out=cc_in[:], in_=sfull[:])
                nc.gpsimd.collective_compute(
                    "AllReduce", ALU.add, replica_groups=groups,
                    ins=[cc_in.opt()], outs=[cc_out.opt()],
                )
                sred = v_pool.tile([B, KO], dt.bfloat16, tag="sred",
                                   name=f"sred{it}", bufs=1)
                nc.gpsimd.dma_start(out=sred[:], in_=cc_out[:])
                # squash: v = s * nrm/((1+nrm)*sqrt(nrm)), nrm = sum_o s^2
                sq1 = v_pool.tile([B, KO], dt.bfloat16, tag="sq1",
                                  name=f"sq1{it}", bufs=1)
                nc.scalar.activation(out=sq1[:], in_=sred[:], func=AF.Square)
                nc.vector.reduce_sum(
                    out=nrm[:],
                    in_=sq1[:].rearrange("p (o k) -> p k o", o=OC),
                    axis=mybir.AxisListType.X,
                )
                nc.vector.tensor_scalar_add(nrm1[:], nrm[:], 1.0)
                nc.scalar.activation(out=den[:], in_=nrm[:], func=AF.Sqrt)
                nc.vector.reciprocal(rden[:], nrm1[:])
                scalb = v_pool.tile([B, K], dt.bfloat16, tag="scalb",
                                    name=f"scalb{it}", bufs=1)
                nc.vector.tensor_mul(scalb[:], den[:], rden[:])
                vbf = v_pool.tile([B, KO], dt.bfloat16, tag="vbf",
                                  name=f"vbf{it}", bufs=1)
                vrep = v_pool.tile([128, KO], dt.bfloat16, tag="vrep",
                                   name=f"vrep{it}", bufs=1)
                nc.vector.tensor_mul(
                    vbf[:].rearrange("p (o k) -> p o k", o=OC),
                    sred[:].rearrange("p (o k) -> p o k", o=OC),
                    scalb[:].unsqueeze(1).broadcast_to((B, OC, K)),
                )
                for r in range(4):
                    eng = nc.gpsimd if r % 2 == 0 else nc.scalar
                    eng.dma_start(
                        out=vrep[r * 32:(r + 1) * 32, :], in_=vbf[:]
                    )
                return vrep

            # ---------- sweep: s0 = (1/K) sum_n priors ---------------------
            s0a = spsum_pool.tile([B, 512], dt.float32, tag="sacc0")
            s0b = spsum_pool.tile([B, 512], dt.float32, tag="sacc1")
            last_wc = None
            for c in range(NCH):
                wc = load_wchunk(c, eng=(nc.sync if c % 2 == 0 else nc.scalar))
                if c == 0:
                    for f in range(2):
                        f0 = f * (NT * 64)
                        eng = nc.sync if f == 0 else nc.scalar
                        eng.dma_start(out=xblk[:, f0:f0 + NT * 64],
                                      in_=xblk_d[:, f0:f0 + NT * 64])
                for tsub in range(4):
                    t = 4 * c + tsub
                    nc.tensor.matmul(
                        out=s0a[:], lhsT=xall[:, t * B:(t + 1) * B],
                        rhs=wc[:, tsub * KO:tsub * KO + 512],
                        start=(t == 0), stop=(t == NT - 1),
                    )
                    nc.tensor.matmul(
                        out=s0b[:], lhsT=xall[:, t * B:(t + 1) * B],
                        rhs=wc[:, tsub * KO + 512:(tsub + 1) * KO],
                        start=(t == 0), stop=(t == NT - 1),
                    )
                last_wc = wc
            vrep = allreduce_squash(s0a, s0b, last=False, it=0)
            # during the s0 AllReduce window: materialize 14,15 from the
            # still-resident last chunk, then 0,1 from a reloaded chunk 0.
            materialize(14, last_wc, 0)
            materialize(15, last_wc, 1)
            wc0 = load_wchunk(0)
            materialize(0, wc0, 0)
            materialize(1, wc0, 1)

            # ---------- passes B (iter1) and C (iter2) ----------------------
            for it in (1, 2):
                sa = spsum_pool.tile([B, 512], dt.float32, tag="sacc0")
                sb = spsum_pool.tile([B, 512], dt.float32, tag="sacc1")
                wc = None

                for st in range(NST):
                    psb = ptiles[st]
                    # tt = P * vrep; tree reduce over o (outer of (o,k))
                    tt = t_pool.tile([128, 4 * KO], dt.bfloat16, tag="t",
                                     name=f"t{it}_{st}")
                    nc.vector.tensor_mul(
                        tt[:].rearrange("p (g f) -> p g f", g=4),
                        psb[:].rearrange("p (g f) -> p g f", g=4),
                        vrep[:].unsqueeze(1).broadcast_to((128, 4, KO)),
                    )
                    t4 = tt[:].rearrange("p (g o k) -> p g o k", g=4, o=OC)
                    u1 = u_pool.tile([128, 4 * 16 * K], dt.bfloat16, tag="u1",
                                     name=f"u1_{it}_{st}")
                    u1v = u1[:].rearrange("p (g o k) -> p g o k", g=4, o=16)
                    nc.vector.tensor_add(u1v, t4[:, :, 0:16, :], t4[:, :, 16:32, :])
                    u2 = u_pool.tile([128, 4 * 8 * K], dt.bfloat16, tag="u2",
                                     name=f"u2_{it}_{st}")
                    u2v = u2[:].rearrange("p (g o k) -> p g o k", g=4, o=8)
                    nc.vector.tensor_add(u2v, u1v[:, :, 0:8, :], u1v[:, :, 8:16, :])
                    u3 = u_pool.tile([128, 4 * 4 * K], dt.bfloat16, tag="u3",
                                     name=f"u3_{it}_{st}")
                    u3v = u3[:].rearrange("p (g o k) -> p g o k", g=4, o=4)
                    nc.vector.tensor_add(u3v, u2v[:, :, 0:4, :], u2v[:, :, 4:8, :])
                    u4 = u_pool.tile([128, 4 * 2 * K], dt.bfloat16, tag="u4",
                                     name=f"u4_{it}_{st}")
                    u4v = u4[:].rearrange("p (g o k) -> p g o k", g=4, o=2)
                    nc.vector.tensor_add(u4v, u3v[:, :, 0:2, :], u3v[:, :, 2:4, :])
                    lt4 = ltiles[st][:].rearrange("p (g o k) -> p g o k",
                                                  g=4, o=1)
                    if it == 1:
                        nc.vector.tensor_add(
                            lt4, u4v[:, :, 0:1, :], u4v[:, :, 1:2, :])
                    else:
                        dtmp = sm_pool.tile([128, 4 * K], dt.float32, tag="dtmp",
                                            name=f"dtmp{it}_{st}")
                        nc.vector.tensor_add(
                            dtmp[:].rearrange("p (g o k) -> p g o k", g=4, o=1),
                            u4v[:, :, 0:1, :], u4v[:, :, 1:2, :])
                        nc.vector.tensor_add(ltiles[st][:], ltiles[st][:],
                                             dtmp[:])
                    # exp per group; Z comes free via accum_out
                    eexp = e_pool.tile([128, 4 * K], dt.bfloat16, tag="eexp",
                                       name=f"eexp{it}_{st}")
                    zacc = sm_pool.tile([128, 4], dt.float32, tag="zacc",
                                        name=f"zacc{it}_{st}")
                    for g in range(4):
                        nc.scalar.activation(
                            out=eexp[:, g * K:(g + 1) * K],
                            in_=ltiles[st][:, g * K:(g + 1) * K],
                            func=AF.Exp, accum_out=zacc[:, g:g + 1])
                    zr = sm_pool.tile([128, 4], dt.float32, tag="zr",
                                      name=f"zr{it}_{st}")
                    nc.vector.reciprocal(zr[:], zacc[:])
                    zblks = []
                    for g in range(4):
                        zblk = sm_pool.tile([128, B], dt.bfloat16, tag="zblk",
                                            name=f"zblk{it}_{st}_{g}")
                        nc.scalar.activation(
                            out=zblk[:], in_=onesblk[:], func=AF.Copy,
                            scale=zr[:, g:g + 1])
                        zblks.append(zblk)
                    # wp = exp * P (unnormalized); 1/Z folded into zblk.
                    wp = wp_pool.tile([128, 4 * KO], dt.bfloat16, tag="wp",
                                      name=f"wp{it}_{st}")
                    wpv = wp[:].rearrange("p (g o k) -> p g o k", g=4, o=OC)
                    psv = psb[:].rearrange("p (g o k) -> p g o k", g=4, o=OC)
                    eev = (eexp[:].rearrange("p (g k) -> p g k", g=4)
                           .unsqueeze(2).broadcast_to((128, 4, OC, K)))
                    nhalf = 2 if st == NST - 1 else 1
                    for hh in range(nhalf):
                        g0, g1 = (0, 4) if nhalf == 1 else (2 * hh, 2 * hh + 2)
                        nc.vector.tensor_mul(
                            wpv[:, g0:g1], psv[:, g0:g1], eev[:, g0:g1])
                        # pass B: interleave materialization of supertile
                        # st+2 (2-slot lookahead; chunk (st+2)//2 loaded on
                        # even slots).
                        if it == 1 and hh == 0 and st + 2 < NST - 2:
                            mt = st + 2
                            if mt % 2 == 0:
                                wc = load_wchunk(mt // 2)
                            materialize(mt, wc, mt % 2)
                        for g in range(g0, g1):
                            gg = 4 * st + g
                            nc.tensor.matmul(
                                out=sa[:], lhsT=zblks[g][:],
                                rhs=wp[:, g * KO:g * KO + 512],
                                start=(gg == 0), stop=(gg == NGRP - 1),
                                skip_group_check=True,
                            )
                            nc.tensor.matmul(
                                out=sb[:], lhsT=zblks[g][:],
                                rhs=wp[:, g * KO + 512:(g + 1) * KO],
                                start=(gg == 0), stop=(gg == NGRP - 1),
                                skip_group_check=True,
                            )
                vrep = allreduce_squash(sa, sb, last=(it == 2), it=it)

    nc.compile()
    return nc


def _prep_inputs(x, route_weights):
    """Host-side shard + layout prep. Returns per-core in_maps.

    SBUF row layout (partition p = s*64 + j*16 + i, s in 2, j in 4, i in 16)
    matches between wmov/xblk/xall.  W free layout is (o,k): col = o*K + k.
    """
    bf16 = ml_dtypes.bfloat16
    xw = x.astype(np.float32)
    W = route_weights.astype(np.float32)
    in_maps = []
    for c in range(NCORES):
        n0 = c * NLOC
        xc = xw[:, n0:n0 + NLOC, :]          # [B, 256, IC]
        Wc = W[n0:n0 + NLOC]                 # [256, K, IC, OC]
        # wmov[s*64+j*16+i, t*KO + o*K + k] = W[8t+4s+j, k, i, o]
        wm = Wc.reshape(NT, 2, 4, K, IC, OC)       # [t, s, j, k, i, o]
        wm = wm.transpose(1, 2, 4, 0, 5, 3)        # [s, j, i, t, o, k]
        wmov = np.ascontiguousarray(
            wm.reshape(128, NT * KO)).astype(bf16)
        # xblk[s*64 + j*16 + i, t*128 + j'*32 + b] = x[b, 8t+4s+j, i]*(j==j')
        xg = xc.transpose(1, 2, 0).reshape(NT, 2, 4, IC, B)  # [t,s,j,i,b]
        xb = np.zeros((2, 4, IC, NT, 4, B), np.float32)      # [s,j,i,t,j',b]
        xgt = xg.transpose(1, 2, 3, 0, 4)                    # [s,j,i,t,b]
        for j in range(4):
            xb[:, j, :, :, j, :] = xgt[:, j]
        xblk = np.ascontiguousarray(
            xb.reshape(128, NT * 128)).astype(bf16)
        # xall[s*64+j*16+i, t*B + b] = x[b, n, i] / K
        xall = np.ascontiguousarray(
            (xgt / K).reshape(128, NT * B)).astype(bf16)
        # ones: delta(b,b')
        ones = np.zeros((128, B), np.float32)
        for j in range(4):
            ones[j * 32 + np.arange(32), np.arange(32)] = 1.0
        onesblk = ones.astype(bf16)
        in_maps.append({
            "wmov": wmov, "xblk": xblk, "xall": xall, "onesblk": onesblk,
        })
    return in_maps


def _get_nc():
    if "nc" not in _CACHE:
        _CACHE["nc"] = _build_bass()
    return _CACHE["nc"]


def kernel(x, route_weights, _trace=False, _trace_kwargs=None):
    from concourse.bass_utils import run_bass_kernel_spmd

    nc = _get_nc()
    in_maps = _prep_inputs(np.asarray(x), np.asarray(route_weights))
    res = run_bass_kernel_spmd(
        nc, in_maps, core_ids=list(range(NCORES)),
        trace=_trace, **(_trace_kwargs or {}),
    )
    # gather/unshard: vout holds each core's local s2 partial [B, (o,k)];
    # sum over cores, then squash on host.
    s = np.zeros((B, KO), np.float64)
    for r in res.results:
        s += r["vout"].astype(np.float64)
    s = s.reshape(B, OC, K).transpose(0, 2, 1)            # [B, K, OC]
    sq = np.sum(s * s, axis=-1, keepdims=True)
    v = (sq / (1.0 + sq)) * s / np.sqrt(sq)
    full = v.astype(np.float32).reshape(B, 1, K, 1, OC)
    if _trace:
        return full, res
    return full


# revision 13
# speedup vs baseline: 1.0599x; 1.0599x over previous
"""CapsuleLayer routing kernel for 8 Trainium2 NeuronCores.

Problem (full shapes): x [B=32, N=2048, IC=16] fp32,
route_weights [N=2048, K=32, IC=16, OC=32] fp32.
  priors = einsum('bni,nkio->bnko', x, W)
  3 routing iterations (softmax over K, weighted sum over N, squash)
  output = squash(s2) shaped [B, 1, K, 1, OC].

Sharding: N (nodes) sharded 8 ways (256 nodes/core).  Cross-core
traffic: one bf16 AllReduce of s [B, K*OC] (64KB) per non-final
iteration; the final iteration's local fp32 s2 partial is DMAed out and
the host sums the 8 partials + applies squash (gather/unshard of the
sum-sharded result).

v5 structure (measured-informed):
  - W is STREAMED in 8 x 1MB bf16 chunks alternating between the SP
    and ACT DMA queues (two queues overlap transfers; ~300GB/s
    aggregate measured vs ~184 single-queue).  The chunk sweep feeds
    the s0 matmuls; the s0 AllReduce fires right after the last chunk.
  - priors are materialized ONCE into 16 persistent SBUF tiles
    (128KB/partition, bf16): supertiles 14,15 during the s0-AllReduce
    window from the still-resident last chunk, 0,1 from a reload, the
    rest interleaved into pass B with 2-slot lookahead.  Pass C reads
    priors straight from SBUF (DVE-bound, no PE cost).
  - DVE per-supertile slot (the hard wall, ~0.555ns/elem tensor_tensor
    in 2x mode; TENSOR_REDUCE measured SLOWER at 1.1-1.8ns/elem, so
    the 4-level bf16 add-tree stays): tt (P*vrep) + tree + 1/Z
    reciprocal + wp (exp*P).  Z comes free from the exp ACTIVATE's
    accum_out (per group).
"""

import numpy as np
import ml_dtypes

B, NLOC, K, IC, OC = 32, 256, 32, 16, 32
NCORES = 8
N = NLOC * NCORES
KO = K * OC            # 1024
NT = NLOC // 8         # 32 sub-tiles of 8 nodes
NST = NLOC // 16       # 16 supertiles of 16 nodes (4 groups of 4)
NGRP = NLOC // 4       # 64 groups of 4 nodes
NCH = 8                # W chunks (2 supertiles per chunk)

_CACHE = {}


def _build_bass():
    import concourse.bass as bass
    import concourse.mybir as mybir
    from concourse import bacc, tile

    dt = mybir.dt
    AF = mybir.ActivationFunctionType
    ALU = mybir.AluOpType

    nc = bacc.Bacc("TRN2", target_bir_lowering=False)

    wmov_d = nc.declare_dram_parameter("wmov", [128, NT * KO], dt.bfloat16, isOutput=False)
    xblk_d = nc.declare_dram_parameter("xblk", [128, NT * 128], dt.bfloat16, isOutput=False)
    xall_d = nc.declare_dram_parameter("xall", [128, NT * B], dt.bfloat16, isOutput=False)
    ones_d = nc.declare_dram_parameter("onesblk", [128, B], dt.bfloat16, isOutput=False)
    vout_d = nc.declare_dram_parameter("vout", [B, KO], dt.float32, isOutput=True)

    groups = [list(range(NCORES))]
    CHW = 4 * KO  # W chunk: 2 supertiles (4 subtiles)

    with tile.TileContext(nc) as tc:
        with (
            tc.tile_pool(name="wch", bufs=2) as wpool,
            tc.tile_pool(name="persist", bufs=1) as ppool,
            tc.tile_pool(name="ptile", bufs=1) as p_pool,
            tc.tile_pool(name="ltiles", bufs=1) as lpool,
            tc.tile_pool(name="tsb", bufs=1) as t_pool,
            tc.tile_pool(name="tree", bufs=1) as u_pool,
            tc.tile_pool(name="wp", bufs=2) as wp_pool,
            tc.tile_pool(name="eexp", bufs=1) as e_pool,
            tc.tile_pool(name="sm", bufs=2) as sm_pool,
            tc.tile_pool(name="vv", bufs=1) as v_pool,
            tc.tile_pool(name="ppsum", bufs=3, space="PSUM") as ppsum_pool,
            tc.tile_pool(name="spsum", bufs=1, space="PSUM") as spsum_pool,
            tc.tile_pool(name="dram", bufs=4, space="DRAM") as dram_pool,
        ):
            xblk = ppool.tile([128, NT * 128], dt.bfloat16, tag="xblk")
            xall = ppool.tile([128, NT * B], dt.bfloat16, tag="xall")
            onesblk = ppool.tile([128, B], dt.bfloat16, tag="ones")

            # CC warmup: tiny AllReduce issued first (content irrelevant),
            # absorbs the ~48us collective-stack cold start under the loads.
            warm_in = dram_pool.tile([B, KO], dt.bfloat16, tag="warmin")
            warm_out = dram_pool.tile([B, KO], dt.bfloat16, tag="warmout")
            nc.gpsimd.collective_compute(
                "AllReduce", ALU.add, replica_groups=groups,
                ins=[warm_in.opt()], outs=[warm_out.opt()],
            )

            nc.sync.dma_start(out=xall[:], in_=xall_d[:])
            nc.scalar.dma_start(out=onesblk[:], in_=ones_d[:])

            # persistent priors tiles, one [128(4n,32b), 4*KO] per supertile
            ptiles = [p_pool.tile([128, 4 * KO], dt.bfloat16, tag=f"P{t}",
                                  name=f"P{t}") for t in range(NST)]
            # persistent logits tiles, one [128(4n,32b), 4*K] per supertile
            ltiles = [lpool.tile([128, 4 * K], dt.float32, tag=f"L{t}",
                                 name=f"L{t}") for t in range(NST)]

            def load_wchunk(c, eng=None):
                wc = wpool.tile([128, CHW], dt.bfloat16, tag="wc",
                                name=f"wc{c}")
                (eng or nc.sync).dma_start(
                    out=wc[:], in_=wmov_d[:, c * CHW:(c + 1) * CHW])
                return wc

            def materialize(st, wc, half):
                """pp matmuls + psb copy for supertile st.

                wc holds chunk st//2; half = st % 2 selects its supertile.
                """
                for tsub in (0, 1):
                    t = 2 * st + tsub
                    for s in (0, 1):
                        g = 2 * tsub + s
                        pp = ppsum_pool.tile(
                            [128, KO], dt.float32, tag="pp",
                            name=f"pp{st}_{g}")
                        lhs = xblk[s * 64:(s + 1) * 64,
                                   t * 128:(t + 1) * 128]
                        co = (2 * half + tsub) * KO
                        for h in (0, 1):
                            nc.tensor.matmul(
                                out=pp[:, h * 512:(h + 1) * 512], lhsT=lhs,
                                rhs=wc[s * 64:(s + 1) * 64,
                                       co + h * 512:co + (h + 1) * 512],
                                start=True, stop=True,
                                skip_group_check=True,
                            )
                        nc.scalar.copy(
                            out=ptiles[st][:, g * KO:(g + 1) * KO], in_=pp[:])

            def allreduce_squash(s_ps0, s_ps1, last, it):
                """PSUM s halves -> AllReduce(bf16) -> squash -> vrep tile.

                Free layout everywhere is (o,k): s[b, o*K + k].
                On the last iteration the local fp32 partial is DMAed out
                instead: the host sums the 8 partials and applies squash
                (gather/unshard of the sum-sharded result).
                """
                if last:
                    sh0 = v_pool.tile([B, 512], dt.float32, tag="sfh",
                                      name=f"sfh{it}_0", bufs=1)
                    nc.scalar.copy(out=sh0[:], in_=s_ps0[:])
                    nc.gpsimd.dma_start(out=vout_d[:, 0:512], in_=sh0[:])
                    sh1 = v_pool.tile([B, 512], dt.float32, tag="sfh",
                                      name=f"sfh{it}_1", bufs=1)
                    nc.scalar.copy(out=sh1[:], in_=s_ps1[:])
                    nc.gpsimd.dma_start(out=vout_d[:, 512:1024], in_=sh1[:])
                    return None
                sfull = v_pool.tile([B, KO], dt.bfloat16, tag="sfull",
                                    name=f"sfull{it}", bufs=1)
                nrm = v_pool.tile([B, K], dt.float32, tag="nrm",
                                  name=f"nrm{it}", bufs=1)
                nrm1 = v_pool.tile([B, K], dt.float32, tag="nrm1",
                                   name=f"nrm1{it}", bufs=1)
                den = v_pool.tile([B, K], dt.float32, tag="den",
                                  name=f"den{it}", bufs=1)
                rden = v_pool.tile([B, K], dt.float32, tag="rden",
                                   name=f"rden{it}", bufs=1)
                nc.vector.tensor_copy(sfull[:, 0:512], s_ps0[:])
                nc.scalar.copy(out=sfull[:, 512:1024], in_=s_ps1[:])
                cc_in = dram_pool.tile([B, KO], dt.bfloat16, tag="ccin")
                cc_out = dram_pool.tile([B, KO], dt.bfloat16, tag="ccout")
                nc.gpsimd.dma_start(out=cc_in[:], in_=sfull[:])
                nc.gpsimd.collective_compute(
                    "AllReduce", ALU.add, replica_groups=groups,
                    ins=[cc_in.opt()], outs=[cc_out.opt()],
                )
                sred = v_pool.tile([B, KO], dt.bfloat16, tag="sred",
                                   name=f"sred{it}", bufs=1)
                nc.gpsimd.dma_start(out=sred[:], in_=cc_out[:])
                # squash: v = s * nrm/((1+nrm)*sqrt(nrm)), nrm = sum_o s^2
                sq1 = v_pool.tile([B, KO], dt.bfloat16, tag="sq1",
                                  name=f"sq1{it}", bufs=1)
                nc.scalar.activation(out=sq1[:], in_=sred[:], func=AF.Square)
                nc.vector.reduce_sum(
                    out=nrm[:],
                    in_=sq1[:].rearrange("p (o k) -> p k o", o=OC),
                    axis=mybir.AxisListType.X,
                )
                nc.vector.tensor_scalar_add(nrm1[:], nrm[:], 1.0)
                nc.scalar.activation(out=den[:], in_=nrm[:], func=AF.Sqrt)
                nc.vector.reciprocal(rden[:], nrm1[:])
                scalb = v_pool.tile([B, K], dt.bfloat16, tag="scalb",
                                    name=f"scalb{it}", bufs=1)
                nc.vector.tensor_mul(scalb[:], den[:], rden[:])
                vbf = v_pool.tile([B, KO], dt.bfloat16, tag="vbf",
                                  name=f"vbf{it}", bufs=1)
                vrep = v_pool.tile([128, KO], dt.bfloat16, tag="vrep",
                                   name=f"vrep{it}", bufs=1)
                nc.vector.tensor_mul(
                    vbf[:].rearrange("p (o k) -> p o k", o=OC),
                    sred[:].rearrange("p (o k) -> p o k", o=OC),
                    scalb[:].unsqueeze(1).broadcast_to((B, OC, K)),
                )
                for r in range(4):
                    eng = nc.gpsimd if r % 2 == 0 else nc.scalar
                    eng.dma_start(
                        out=vrep[r * 32:(r + 1) * 32, :], in_=vbf[:]
                    )
                return vrep

            # ---------- sweep: s0 = (1/K) sum_n priors ---------------------
            s0a = spsum_pool.tile([B, 512], dt.float32, tag="sacc0")
            s0b = spsum_pool.tile([B, 512], dt.float32, tag="sacc1")
            last_wc = None
            for c in range(NCH):
                wc = load_wchunk(c, eng=(nc.sync if c % 2 == 0 else nc.scalar))
                if c == 0:
                    for f in range(2):
                        f0 = f * (NT * 64)
                        eng = nc.sync if f == 0 else nc.scalar
                        eng.dma_start(out=xblk[:, f0:f0 + NT * 64],
                                      in_=xblk_d[:, f0:f0 + NT * 64])
                for tsub in range(4):
                    t = 4 * c + tsub
                    nc.tensor.matmul(
                        out=s0a[:], lhsT=xall[:, t * B:(t + 1) * B],
                        rhs=wc[:, tsub * KO:tsub * KO + 512],
                        start=(t == 0), stop=(t == NT - 1),
                    )
                    nc.tensor.matmul(
                        out=s0b[:], lhsT=xall[:, t * B:(t + 1) * B],
                        rhs=wc[:, tsub * KO + 512:(tsub + 1) * KO],
                        start=(t == 0), stop=(t == NT - 1),
                    )
                last_wc = wc
            vrep = allreduce_squash(s0a, s0b, last=False, it=0)
            # during the s0 AllReduce window: materialize 14,15 from the
            # still-resident last chunk, then 0,1 from a reloaded chunk 0.
            materialize(14, last_wc, 0)
            materialize(15, last_wc, 1)
            wc0 = load_wchunk(0)
            materialize(0, wc0, 0)
            materialize(1, wc0, 1)
            wc1 = load_wchunk(1)
            materialize(2, wc1, 0)
            materialize(3, wc1, 1)

            # ---------- passes B (iter1) and C (iter2) ----------------------
            for it in (1, 2):
                sa = spsum_pool.tile([B, 512], dt.float32, tag="sacc0")
                sb = spsum_pool.tile([B, 512], dt.float32, tag="sacc1")
                wc = None

                for st in range(NST):
                    psb = ptiles[st]
                    # tt = P * vrep; tree reduce over o (outer of (o,k))
                    tt = t_pool.tile([128, 4 * KO], dt.bfloat16, tag="t",
                                     name=f"t{it}_{st}")
                    nc.vector.tensor_mul(
                        tt[:].rearrange("p (g f) -> p g f", g=4),
                        psb[:].rearrange("p (g f) -> p g f", g=4),
                        vrep[:].unsqueeze(1).broadcast_to((128, 4, KO)),
                    )
                    t4 = tt[:].rearrange("p (g o k) -> p g o k", g=4, o=OC)
                    u1 = u_pool.tile([128, 4 * 16 * K], dt.bfloat16, tag="u1",
                                     name=f"u1_{it}_{st}")
                    u1v = u1[:].rearrange("p (g o k) -> p g o k", g=4, o=16)
                    nc.vector.tensor_add(u1v, t4[:, :, 0:16, :], t4[:, :, 16:32, :])
                    u2 = u_pool.tile([128, 4 * 8 * K], dt.bfloat16, tag="u2",
                                     name=f"u2_{it}_{st}")
                    u2v = u2[:].rearrange("p (g o k) -> p g o k", g=4, o=8)
                    nc.vector.tensor_add(u2v, u1v[:, :, 0:8, :], u1v[:, :, 8:16, :])
                    u3 = u_pool.tile([128, 4 * 4 * K], dt.bfloat16, tag="u3",
                                     name=f"u3_{it}_{st}")
                    u3v = u3[:].rearrange("p (g o k) -> p g o k", g=4, o=4)
                    nc.vector.tensor_add(u3v, u2v[:, :, 0:4, :], u2v[:, :, 4:8, :])
                    u4 = u_pool.tile([128, 4 * 2 * K], dt.bfloat16, tag="u4",
                                     name=f"u4_{it}_{st}")
                    u4v = u4[:].rearrange("p (g o k) -> p g o k", g=4, o=2)
                    nc.vector.tensor_add(u4v, u3v[:, :, 0:2, :], u3v[:, :, 2:4, :])
                    lt4 = ltiles[st][:].rearrange("p (g o k) -> p g o k",
                                                  g=4, o=1)
                    if it == 1:
                        nc.vector.tensor_add(
                            lt4, u4v[:, :, 0:1, :], u4v[:, :, 1:2, :])
                    else:
                        dtmp = sm_pool.tile([128, 4 * K], dt.float32, tag="dtmp",
                                            name=f"dtmp{it}_{st}")
                        nc.vector.tensor_add(
                            dtmp[:].rearrange("p (g o k) -> p g o k", g=4, o=1),
                            u4v[:, :, 0:1, :], u4v[:, :, 1:2, :])
                        nc.vector.tensor_add(ltiles[st][:], ltiles[st][:],
                                             dtmp[:])
                    # exp per group; Z comes free via accum_out
                    eexp = e_pool.tile([128, 4 * K], dt.bfloat16, tag="eexp",
                                       name=f"eexp{it}_{st}")
                    zacc = sm_pool.tile([128, 4], dt.float32, tag="zacc",
                                        name=f"zacc{it}_{st}")
                    for g in range(4):
                        nc.scalar.activation(
                            out=eexp[:, g * K:(g + 1) * K],
                            in_=ltiles[st][:, g * K:(g + 1) * K],
                            func=AF.Exp, accum_out=zacc[:, g:g + 1])
                    zr = sm_pool.tile([128, 4], dt.float32, tag="zr",
                                      name=f"zr{it}_{st}")
                    nc.vector.reciprocal(zr[:], zacc[:])
                    zblks = []
                    for g in range(4):
                        zblk = sm_pool.tile([128, B], dt.bfloat16, tag="zblk",
                                            name=f"zblk{it}_{st}_{g}")
                        nc.scalar.activation(
                            out=zblk[:], in_=onesblk[:], func=AF.Copy,
                            scale=zr[:, g:g + 1])
                        zblks.append(zblk)
                    # wp = exp * P (unnormalized); 1/Z folded into zblk.
                    wp = wp_pool.tile([128, 4 * KO], dt.bfloat16, tag="wp",
                                      name=f"wp{it}_{st}")
                    wpv = wp[:].rearrange("p (g o k) -> p g o k", g=4, o=OC)
                    psv = psb[:].rearrange("p (g o k) -> p g o k", g=4, o=OC)
                    eev = (eexp[:].rearrange("p (g k) -> p g k", g=4)
                           .unsqueeze(2).broadcast_to((128, 4, OC, K)))
                    nhalf = 2 if st == NST - 1 else 1
                    for hh in range(nhalf):
                        g0, g1 = (0, 4) if nhalf == 1 else (2 * hh, 2 * hh + 2)
                        nc.vector.tensor_mul(
                            wpv[:, g0:g1], psv[:, g0:g1], eev[:, g0:g1])
                        # pass B: interleave materialization of supertile
                        # st+2 (2-slot lookahead; chunk (st+2)//2 loaded on
                        # even slots).
                        if it == 1 and hh == 0 and 4 <= st + 2 < NST - 2:
                            mt = st + 2
                            if mt % 2 == 0:
                                wc = load_wchunk(mt // 2)
                            materialize(mt, wc, mt % 2)
                        for g in range(g0, g1):
                            gg = 4 * st + g
                            nc.tensor.matmul(
                                out=sa[:], lhsT=zblks[g][:],
                                rhs=wp[:, g * KO:g * KO + 512],
                                start=(gg == 0), stop=(gg == NGRP - 1),
                                skip_group_check=True,
                            )
                            nc.tensor.matmul(
                                out=sb[:], lhsT=zblks[g][:],
                                rhs=wp[:, g * KO + 512:(g + 1) * KO],
                                start=(gg == 0), stop=(gg == NGRP - 1),
                                skip_group_check=True,
                            )
                vrep = allreduce_squash(sa, sb, last=(it == 2), it=it)

    nc.compile()
    return nc


def _prep_inputs(x, route_weights):
    """Host-side shard + layout prep. Returns per-core in_maps.

    SBUF row layout (partition p = s*64 + j*16 + i, s in 2, j in 4, i in 16)
    matches between wmov/xblk/xall.  W free layout is (o,k): col = o*K + k.
    """
    bf16 = ml_dtypes.bfloat16
    xw = x.astype(np.float32)
    W = route_weights.astype(np.float32)
    in_maps = []
    for c in range(NCORES):
        n0 = c * NLOC
        xc = xw[:, n0:n0 + NLOC, :]          # [B, 256, IC]
        Wc = W[n0:n0 + NLOC]                 # [256, K, IC, OC]
        # wmov[s*64+j*16+i, t*KO + o*K + k] = W[8t+4s+j, k, i, o]
        wm = Wc.reshape(NT, 2, 4, K, IC, OC)       # [t, s, j, k, i, o]
        wm = wm.transpose(1, 2, 4, 0, 5, 3)        # [s, j, i, t, o, k]
        wmov = np.ascontiguousarray(
            wm.reshape(128, NT * KO)).astype(bf16)
        # xblk[s*64 + j*16 + i, t*128 + j'*32 + b] = x[b, 8t+4s+j, i]*(j==j')
        xg = xc.transpose(1, 2, 0).reshape(NT, 2, 4, IC, B)  # [t,s,j,i,b]
        xb = np.zeros((2, 4, IC, NT, 4, B), np.float32)      # [s,j,i,t,j',b]
        xgt = xg.transpose(1, 2, 3, 0, 4)                    # [s,j,i,t,b]
        for j in range(4):
            xb[:, j, :, :, j, :] = xgt[:, j]
        xblk = np.ascontiguousarray(
            xb.reshape(128, NT * 128)).astype(bf16)
        # xall[s*64+j*16+i, t*B + b] = x[b, n, i] / K
        xall = np.ascontiguousarray(
            (xgt / K).reshape(128, NT * B)).astype(bf16)
        # ones: delta(b,b')
        ones = np.zeros((128, B), np.float32)
        for j in range(4):
            ones[j * 32 + np.arange(32), np.arange(32)] = 1.0
        onesblk = ones.astype(bf16)
        in_maps.append({
            "wmov": wmov, "xblk": xblk, "xall": xall, "onesblk": onesblk,
        })
    return in_maps


def _get_nc():
    if "nc" not in _CACHE:
        _CACHE["nc"] = _build_bass()
    return _CACHE["nc"]


def kernel(x, route_weights, _trace=False, _trace_kwargs=None):
    from concourse.bass_utils import run_bass_kernel_spmd

    nc = _get_nc()
    in_maps = _prep_inputs(np.asarray(x), np.asarray(route_weights))
    res = run_bass_kernel_spmd(
        nc, in_maps, core_ids=list(range(NCORES)),
        trace=_trace, **(_trace_kwargs or {}),
    )
    # gather/unshard: vout holds each core's local s2 partial [B, (o,k)];
    # sum over cores, then squash on host.
    s = np.zeros((B, KO), np.float64)
    for r in res.results:
        s += r["vout"].astype(np.float64)
    s = s.reshape(B, OC, K).transpose(0, 2, 1)            # [B, K, OC]
    sq = np.sum(s * s, axis=-1, keepdims=True)
    v = (sq / (1.0 + sq)) * s / np.sqrt(sq)
    full = v.astype(np.float32).reshape(B, 1, K, 1, OC)
    if _trace:
        return full, res
    return full
